# revision 20
# baseline (speedup 1.0000x reference)
"""Trainium2 Bass kernel for GQA attention with RoPE (B=2, S=1024, HID=2048,
16 q heads / 4 kv heads, head dim 128, causal).

Sharding: 8 cores = 2 batches x 4 kv-head groups. Core c = b*4 + g handles
batch b and kv head g (query heads 4g..4g+3). Each core computes a partial
output y_part = attn_heads @ wo_shard; the host sums the 4 partials per batch.

All tensors fp16 on the wire and in SBUF (host casts inputs; host upcasts and
sums the fp16 partials). Matmuls fp16 (1 cyc/row), except the softmax
denominator which runs as fp8e4m3 DoubleRow over chunk pairs (expst8 is a DVE
cast of the fp16 expst). Causal masking is multiplicative-zero on expst via
gpsimd affine_select (Pool engine), so the DVE stays out of the mask path.

Per-core dataflow:
  Phase A (per 128-row chunk g, software-pipelined):
    x chunk --PE transpose--> xT --mm--> q,k,v (natural); batched RoPE on DVE
    (broadcast cos/sin over the 5 q/k blocks); PE transpose q_rope/k_rope ->
    persistent qT[d,h,s], kT[d,s]; v natural -> vv[s,d].
  Attention (per 256-col tile t, head h, 2-stage pipelined; light doses
  interleave into phase A, the bulk runs after):
    scoresT[sk,sq] = kT_chunk.T @ qT ; exp on ACT -> expst f16 ; diagonal
    causal zeroing on Pool ; den = DoubleRow fp8 ones.T @ expst8 ; U^T
    accumulated fp16 ; rec = 1/den (DVE) ; uT = U^T * rec (DVE, f16).
    wo: y[g,:] = sum_h uT_h.T @ wo_h -> y_sb f16 -> DRAM (one DMA per row).
"""

import sys

import numpy as np

for _p in ("/opt/trn_rl_repo", "/root/.axon_site/_ro/trn_rl_repo"):
    if _p not in sys.path:
        sys.path.append(_p)

from contextlib import ExitStack

import concourse.bass as bass
import concourse.mybir as mybir
from concourse import bacc
from concourse.masks import make_identity
from concourse.tile import TileContext

P = 128           # partitions / head dim / seq chunk
S = 1024          # sequence length
HID = 2048        # model dim
NH = 4            # query heads per core
D = 128           # head dim
TQ = 256          # query macro-tile
NT = S // TQ      # 4 macro tiles
KC = HID // P     # 16 contraction chunks
NSK = S // P      # 8 key chunks
NG = S // P       # 8 row chunks
H2 = D // 2
F32 = mybir.dt.float32
F16 = mybir.dt.float16
F8 = mybir.dt.float8e4
SCALE = 1.0 / float(np.sqrt(D))
NEG = -30000.0
AL = mybir.AluOpType
AF = mybir.ActivationFunctionType
DR = mybir.MatmulPerfMode.DoubleRow

# fp8 expst is dead: exp values span e^{+-9.2}, outside e4m3 range (den can
# round to 0 -> 1/den=inf -> NaN; large values saturate at 448)
USE_F8 = False

N_CORES = 8
B = 2
N_KV = 4


def build_nc():
    nc = bacc.Bacc("TRN2", target_bir_lowering=False, debug=False)
    x_d = nc.declare_dram_parameter("x", [S, HID], F16, isOutput=False)
    cos_d = nc.declare_dram_parameter("cos", [S, D], F16, isOutput=False)
    sin_d = nc.declare_dram_parameter("sin", [S, D], F16, isOutput=False)
    wq_d = nc.declare_dram_parameter("wq", [HID, NH * D], F16, isOutput=False)
    wk_d = nc.declare_dram_parameter("wk", [HID, D], F16, isOutput=False)
    wv_d = nc.declare_dram_parameter("wv", [HID, D], F16, isOutput=False)
    wo_d = nc.declare_dram_parameter("wo", [NH * D, HID], F16, isOutput=False)
    out_d = nc.declare_dram_parameter("out", [S, HID], F16, isOutput=True)

    with TileContext(nc) as tc, ExitStack() as ctx:
        consts = ctx.enter_context(tc.tile_pool(name="consts", bufs=1))
        wpool = ctx.enter_context(tc.tile_pool(name="wpool", bufs=1))
        persist = ctx.enter_context(tc.tile_pool(name="persist", bufs=1))

        # ---- tile declarations (DMAs can start before consts are built) ----
        ident_f32 = consts.tile([P, P], F32, tag="ident_f32")
        ident = consts.tile([P, P], F16, tag="ident")
        ones8 = consts.tile([P, 2, P], F8, tag="ones8")
        ones16 = consts.tile([P, P], F16, tag="ones16")

        # ---- weights (partition-chunked layouts) ----
        wq_sb = wpool.tile([P, KC, NH * D], F16, tag="wq")
        wq_r = wq_d[:].rearrange("(c p) n -> p c n", p=P)
        wkv_sb = wpool.tile([P, KC, 2 * D], F16, tag="wkv")
        wo_sb = wpool.tile([P, NH, HID], F16, tag="wo")
        wo_r = wo_d[:].rearrange("(h p) n -> p h n", p=P)
        cos_sb = wpool.tile([P, NG, D], F16, tag="cos")
        sin_sb = wpool.tile([P, NG, D], F16, tag="sin")

        # persistent transposed activations
        qT_all = persist.tile([P, NH, S], F16, tag="qT")   # [d, h, sq]
        kT = persist.tile([P, S], F16, tag="kT")           # [d, sk]
        vv = persist.tile([P, NSK, D], F16, tag="vv")      # v natural [sk, d]

        # ---- SBUF working pools ----
        pa = ctx.enter_context(tc.tile_pool(name="pa", bufs=2))
        pb = ctx.enter_context(tc.tile_pool(name="pb", bufs=2))
        # ---- PSUM (8 banks): phase A uses qkv 2 + tp 2 (inner stack,
        # released before wo); s 2 + ud 2 persist; y 2 allocated after ----
        ps_s = ctx.enter_context(tc.tile_pool(name="ps_s", bufs=2, space="PSUM"))
        ps_ud = ctx.enter_context(tc.tile_pool(name="ps_ud", bufs=2, space="PSUM"))
        phase_a_ctx = ExitStack()
        ps_qkv = phase_a_ctx.enter_context(
            tc.tile_pool(name="ps_qkv", bufs=1, space="PSUM")
        )
        ps_tp = phase_a_ctx.enter_context(
            tc.tile_pool(name="ps_tp", bufs=2, space="PSUM")
        )

        x_tiles = [None] * NG
        x_r = x_d[:].rearrange("(c p) n -> p c n", p=P)

        def emit_xdma_pair(g, single=False):
            """One DMA per chunk pair (fewer per-DMA overheads)."""
            n = 1 if single else 2
            xp = pa.tile([P, n, HID], F16, tag=f"xnat{g}", bufs=1)
            nc.sync.dma_start(out=xp, in_=x_r[:, g : g + n, :])
            for j in range(n):
                x_tiles[g + j] = xp[:, j, :]

        # Three parallel DGE queues: x pairs on sync (SP); wq/wo on Pool;
        # wkv/cos/sin on the scalar (ACT) queue — each queue serializes its
        # own transfers, so spreading streams cuts the critical cadence.
        emit_xdma_pair(0, single=True)
        nc.gpsimd.dma_start(out=wq_sb[:, 0:8, :], in_=wq_r[:, 0:8, :])
        nc.scalar.dma_start(
            out=wkv_sb[:, :, 0:D], in_=wk_d[:].rearrange("(c p) n -> p c n", p=P)
        )
        emit_xdma_pair(1, single=True)
        nc.scalar.dma_start(
            out=wkv_sb[:, :, D : 2 * D],
            in_=wv_d[:].rearrange("(c p) n -> p c n", p=P),
        )
        nc.scalar.dma_start(
            out=cos_sb, in_=cos_d[:].rearrange("(c p) d -> p c d", p=P)
        )
        nc.scalar.dma_start(
            out=sin_sb, in_=sin_d[:].rearrange("(c p) d -> p c d", p=P)
        )
        nc.gpsimd.dma_start(out=wq_sb[:, 8:16, :], in_=wq_r[:, 8:16, :])

        # ---- constants (gpsimd/DVE work overlapping the DMAs) ----
        make_identity(nc, ident_f32)
        nc.vector.tensor_copy(ident, ident_f32)
        nc.vector.memset(ones8, 1.0)
        nc.vector.memset(ones16, 1.0)
        # causal masks for the two diagonal-straddling chunk positions
        m12 = consts.tile([P, 2 * TQ], F32, tag="m12")
        nc.gpsimd.memset(m12, 0.0)
        nc.gpsimd.affine_select(
            out=m12[:, 0:TQ], in_=m12[:, 0:TQ], compare_op=AL.is_ge, fill=NEG,
            base=0, pattern=[[1, TQ]], channel_multiplier=-1,
        )
        nc.gpsimd.affine_select(
            out=m12[:, TQ : 2 * TQ], in_=m12[:, TQ : 2 * TQ],
            compare_op=AL.is_ge, fill=NEG,
            base=-P, pattern=[[1, TQ]], channel_multiplier=-1,
        )

        # warm up the PE clock while the first DMAs are in flight
        warm_ps = ps_s.tile([P, 2 * TQ], F32, tag="s", name="warm")
        for _ in range(30):
            nc.tensor.matmul(warm_ps[:, 0:P], ident, ident, start=True, stop=True)
        warm_drain = pa.tile([P, 4], F32, tag="warmdrain", bufs=1)
        nc.vector.tensor_copy(warm_drain, warm_ps[:, 0:4])

        emit_xdma_pair(2)
        emit_xdma_pair(4)
        emit_xdma_pair(6)
        wo_next = [0]

        def emit_wo_dma():
            h = wo_next[0]
            if h < NH:
                nc.gpsimd.dma_start(
                    out=wo_sb[:, h : h + 2, :], in_=wo_r[:, h : h + 2, :]
                )
                wo_next[0] += 2

        def transposes(g):
            """x chunk -> xT (PE transpose, f16)."""
            x_nat = x_tiles[g]
            xT = pa.tile([P, KC, P], F16, tag="xT", bufs=2)
            xT_flat = xT.rearrange("p c d -> p (c d)")
            for kb in range(KC // 4):
                tp_ps = ps_tp.tile([P, 5 * P], F16, tag="tp", name="tp")
                for j in range(4):
                    k = 4 * kb + j
                    nc.tensor.transpose(
                        tp_ps[:, j * P : (j + 1) * P],
                        x_nat[:, k * P : (k + 1) * P],
                        ident,
                    )
                if kb % 2 == 0:
                    nc.vector.tensor_copy(
                        xT_flat[:, kb * 4 * P : (kb + 1) * 4 * P], tp_ps[:, 0 : 4 * P]
                    )
                else:
                    nc.scalar.activation(
                        out=xT_flat[:, kb * 4 * P : (kb + 1) * 4 * P],
                        in_=tp_ps[:, 0 : 4 * P],
                        func=AF.Copy,
                    )
            return xT

        def proj(g, xT):
            """q, k, v projections for chunk g (PE, accumulating in PSUM)."""
            qkv_ps = ps_qkv.tile([P, NH * D + 2 * D], F32, tag="qkv")
            q_ps = qkv_ps[:, 0 : NH * D]
            kv_ps = qkv_ps[:, NH * D : NH * D + 2 * D]
            for k in range(KC):
                nc.tensor.matmul(
                    q_ps, xT[:, k, :], wq_sb[:, k, :],
                    start=(k == 0), stop=(k == KC - 1),
                )
            for k in range(KC):
                nc.tensor.matmul(
                    kv_ps, xT[:, k, :], wkv_sb[:, k, :],
                    start=(k == 0), stop=(k == KC - 1),
                )
            # copy-out split across ACT (q) and DVE (kv) to free the bank fast
            qkv_sb = pa.tile([P, NH * D + 2 * D], F16, tag="qkvsb")
            nc.scalar.activation(
                out=qkv_sb[:, 0 : NH * D], in_=q_ps, func=AF.Copy
            )
            nc.vector.tensor_copy(qkv_sb[:, NH * D :], kv_ps)
            return qkv_sb

        def rope_stage(g, qkv_sb):
            """Batched RoPE over the 5 q/k blocks (DVE, broadcast cos/sin)."""
            qk = qkv_sb[:, 0 : 5 * D].rearrange("p (f d) -> p f d", d=D)
            sin_lo = sin_sb[:, g : g + 1, 0:H2].to_broadcast((P, 5, H2))
            sin_hi = sin_sb[:, g : g + 1, H2:D].to_broadcast((P, 5, H2))
            cos_bc = cos_sb[:, g : g + 1, :].to_broadcast((P, 5, D))
            tmp = pa.tile([P, 5, D], F16, tag="ropetmp")
            dst = pa.tile([P, 5, D], F16, tag="qkrope")
            nc.vector.scalar_tensor_tensor(
                out=tmp[:, :, 0:H2], in0=qk[:, :, H2:D], scalar=-1.0,
                in1=sin_lo, op0=AL.mult, op1=AL.mult,
            )
            nc.vector.tensor_tensor(
                out=tmp[:, :, H2:D], in0=qk[:, :, 0:H2], in1=sin_hi, op=AL.mult
            )
            nc.vector.tensor_tensor(out=dst, in0=qk, in1=cos_bc, op=AL.mult)
            nc.vector.tensor_tensor(
                out=dst.rearrange("p f d -> p (f d)"),
                in0=dst.rearrange("p f d -> p (f d)"),
                in1=tmp.rearrange("p f d -> p (f d)"),
                op=AL.add,
            )
            # v copy-out (cast f16)
            nc.vector.tensor_copy(vv[:, g, :], qkv_sb[:, 5 * D : 6 * D])
            return dst

        def rope_transpose(g, dst):
            """Transpose RoPE'd q/k into persistent qT_all / kT."""
            tq_ps = ps_tp.tile([P, 5 * P], F16, tag="tp", name="tq")
            for f in range(5):
                nc.tensor.transpose(
                    tq_ps[:, f * P : (f + 1) * P], dst[:, f, :], ident
                )
            nc.vector.tensor_copy(
                qT_all[:, :, g * P : (g + 1) * P],
                tq_ps[:, 0 : 4 * P].rearrange("p (h d) -> p h d", h=NH),
            )
            nc.scalar.activation(
                out=kT[:, g * P : (g + 1) * P], in_=tq_ps[:, 4 * P : 5 * P],
                func=AF.Copy,
            )

        ropes = [None] * NG
        pend = [None] * NG

        def emit_phase_a(g):
            if g >= 2:
                gg = g - 2
                with nc.named_scope(f"rope_{gg}"):
                    ropes[gg] = rope_stage(gg, pend[gg][1])
            if g < NG:
                if g in (2, 3):
                    emit_wo_dma()
                with nc.named_scope(f"tp_{g}"):
                    xT = transposes(g)
                pend[g] = [xT, None]
            if g >= 1 and g - 1 < NG:
                gg = g - 1
                with nc.named_scope(f"proj_{gg}"):
                    qkv_sb = proj(gg, pend[gg][0])
                pend[gg][1] = qkv_sb
            if g >= 2:
                gg = g - 2
                with nc.named_scope(f"ropeT_{gg}"):
                    rope_transpose(gg, ropes[gg])
                pend[gg] = None

        # ---------- attention ----------
        EDT = F8 if USE_F8 else F16

        def scores_head(t, h):
            """scoresT + causal mask (diagonal, DVE pre-exp) + exp -> expst.

            expst is fp8: the SAME quantized values feed both the PV
            numerator (mixed fp16xfp8 matmul) and the denominator (fp8
            DoubleRow), so quantization error cancels in the ratio."""
            qT_h = qT_all[:, h, t * TQ : (t + 1) * TQ]
            expst = pb.tile([P, NSK, TQ], EDT, tag="expst", bufs=3)
            expst_flat = expst.rearrange("p c f -> p (c f)")
            for pi in range(t + 1):
                s_ps = ps_s.tile([P, 2 * TQ], F32, tag="s", name="s")
                for half in range(2):
                    ik = 2 * pi + half
                    nc.tensor.matmul(
                        s_ps[:, half * TQ : (half + 1) * TQ],
                        kT[:, ik * P : (ik + 1) * P], qT_h,
                        start=True, stop=True,
                    )
                if pi == t:
                    nc.vector.tensor_tensor(
                        out=s_ps, in0=s_ps, in1=m12, op=AL.add
                    )
                nc.scalar.activation(
                    out=expst_flat[:, pi * 2 * TQ : (pi + 1) * 2 * TQ],
                    in_=s_ps, func=AF.Exp, scale=SCALE,
                )
            return expst

        def dnpv_head(t, h, expst, uT_t):
            """denominator + PV matmuls, then normalize into uT_t (DVE)."""
            nsk = 2 * (t + 1)
            ud_ps = ps_ud.tile([P, 2 * TQ], F32, tag="ud", name="ud")
            u_ps = ud_ps[:, 0:TQ]
            den_ps = ud_ps[:, TQ : 2 * TQ]
            if USE_F8:
                for pi in range(t + 1):
                    nc.tensor.matmul(
                        den_ps, ones8,
                        expst[:, 2 * pi : 2 * pi + 2, :],
                        start=(pi == 0), stop=(pi == t), perf_mode=DR,
                    )
            else:
                for ik in range(nsk):
                    nc.tensor.matmul(
                        den_ps, ones16, expst[:, ik, :],
                        start=(ik == 0), stop=(ik == nsk - 1),
                    )
            rec = pb.tile([P, TQ], F32, tag="rec", bufs=2)
            nc.vector.reciprocal(rec, den_ps)
            for ik in range(nsk):
                nc.tensor.matmul(
                    u_ps, vv[:, ik, :], expst[:, ik, :],
                    start=(ik == 0), stop=(ik == nsk - 1),
                )
            nc.vector.tensor_tensor(
                out=uT_t[:, h, :], in0=u_ps, in1=rec, op=AL.mult
            )

        ps_y_box = [None]

        def wo_stage(t, uT_t):
            for sub in range(2):
                g = 2 * t + sub
                y_sb = pb.tile([P, HID], F16, tag="ysb", bufs=2)
                for n in range(HID // 512):
                    y_ps = ps_y_box[0].tile([P, 512], F32, tag="y", name="y")
                    for h in range(NH):
                        nc.tensor.matmul(
                            y_ps,
                            uT_t[:, h, sub * P : (sub + 1) * P],
                            wo_sb[:, h, n * 512 : (n + 1) * 512],
                            start=(h == 0), stop=(h == NH - 1),
                        )
                    if n % 2 == 0:
                        nc.vector.tensor_copy(
                            y_sb[:, n * 512 : (n + 1) * 512], y_ps
                        )
                    else:
                        nc.scalar.activation(
                            out=y_sb[:, n * 512 : (n + 1) * 512], in_=y_ps,
                            func=AF.Copy,
                        )
                nc.gpsimd.dma_start(
                    out=out_d[g * P : (g + 1) * P, :], in_=y_sb
                )

        steps = [(t, h) for t in range(NT) for h in range(NH)]
        uts = {}
        att_i = [0]
        pending_wo = []

        def emit_attention_step(defer_wo):
            i = att_i[0]
            if i >= len(steps) + 2:
                return False
            if i < len(steps):
                t, h = steps[i]
                if h == 0:
                    uts[t] = pb.tile([P, NH, TQ], F16, tag="uT", name=f"uT{t}", bufs=4)
                with nc.named_scope(f"sc_{t}_{h}"):
                    uts[(t, h)] = scores_head(t, h)
            if 1 <= i < len(steps) + 1:
                t, h = steps[i - 1]
                with nc.named_scope(f"dnpv_{t}_{h}"):
                    dnpv_head(t, h, uts.pop((t, h)), uts[t])
            if i >= 2 and (i - 2) % NH == NH - 1:
                t = steps[i - 2][0]
                if defer_wo:
                    pending_wo.append(t)
                else:
                    while pending_wo:
                        tp = pending_wo.pop(0)
                        with nc.named_scope(f"wo_{tp}"):
                            wo_stage(tp, uts.pop(tp))
                    with nc.named_scope(f"wo_{t}"):
                        wo_stage(t, uts.pop(t))
            att_i[0] += 1
            return True

        def att_ready():
            i = att_i[0]
            if i >= len(steps) + 2:
                return False
            if i < len(steps):
                t, _h = steps[i]
                if 2 * t + 1 > done_g[0]:
                    return False
            return True

        # drive: phase A strictly prioritized; 1 attention step per
        # iteration to fill PE bubbles (wo deferred), the bulk after
        done_g = [-1]
        for g in range(NG + 2):
            emit_phase_a(g)
            done_g[0] = g - 2
            if g >= 3 and att_ready():
                emit_attention_step(defer_wo=True)
        # phase A fully emitted: release its PSUM banks, give wo its own
        phase_a_ctx.close()
        ps_y_box[0] = ctx.enter_context(
            tc.tile_pool(name="ps_y", bufs=2, space="PSUM")
        )
        while pending_wo and att_i[0] > 2:
            tp_ = pending_wo.pop(0)
            with nc.named_scope(f"wo_{tp_}"):
                wo_stage(tp_, uts.pop(tp_))
        while emit_attention_step(defer_wo=False):
            pass

    nc.compile()
    return nc


def shard_inputs(x, cos, sin, wq, wk, wv, wo):
    """Build per-core input maps (fp16): core = b*4 + g."""
    f16 = np.float16
    in_maps = []
    for c in range(N_CORES):
        b, g = divmod(c, N_KV)
        in_maps.append(
            {
                "x": np.ascontiguousarray(x[b], dtype=f16),
                "cos": np.ascontiguousarray(cos, dtype=f16),
                "sin": np.ascontiguousarray(sin, dtype=f16),
                "wq": np.ascontiguousarray(
                    wq[:, g * NH * D : (g + 1) * NH * D], dtype=f16
                ),
                "wk": np.ascontiguousarray(wk[:, g * D : (g + 1) * D], dtype=f16),
                "wv": np.ascontiguousarray(wv[:, g * D : (g + 1) * D], dtype=f16),
                "wo": np.ascontiguousarray(
                    wo[g * NH * D : (g + 1) * NH * D, :], dtype=f16
                ),
            }
        )
    return in_maps


_NC_CACHE = {}


def get_nc():
    if "nc" not in _NC_CACHE:
        _NC_CACHE["nc"] = build_nc()
    return _NC_CACHE["nc"]


def kernel(x, cos, sin, wq, wk, wv, wo, _trace=False):
    from concourse.bass_utils import run_bass_kernel_spmd

    x = np.asarray(x, dtype=np.float32)
    cos = np.asarray(cos, dtype=np.float32)
    sin = np.asarray(sin, dtype=np.float32)
    wq = np.asarray(wq, dtype=np.float32)
    wk = np.asarray(wk, dtype=np.float32)
    wv = np.asarray(wv, dtype=np.float32)
    wo = np.asarray(wo, dtype=np.float32)

    nc = get_nc()
    in_maps = shard_inputs(x, cos, sin, wq, wk, wv, wo)
    res = run_bass_kernel_spmd(nc, in_maps, list(range(N_CORES)), trace=_trace)
    parts = [
        np.asarray(res.results[c]["out"], dtype=np.float32) for c in range(N_CORES)
    ]
    y = np.stack(
        [sum(parts[b * N_KV + g] for g in range(N_KV)) for b in range(B)], axis=0
    )
    if _trace:
        kernel.last_result = res
    return y


# revision 23
# speedup vs baseline: 1.0879x; 1.0879x over previous
"""Trainium2 Bass kernel for GQA attention with RoPE (B=2, S=1024, HID=2048,
16 q heads / 4 kv heads, head dim 128, causal).

Sharding: 8 cores = 2 batches x 4 kv-head groups. Core c = b*4 + g handles
batch b and kv head g (query heads 4g..4g+3). Each core computes a partial
output y_part = attn_heads @ wo_shard; the host sums the 4 partials per batch.

All tensors fp16 on the wire and in SBUF (host casts inputs; host upcasts and
sums the fp16 partials). Matmuls fp16 (1 cyc/row), except the softmax
denominator which runs as fp8e4m3 DoubleRow over chunk pairs (expst8 is a DVE
cast of the fp16 expst). Causal masking is multiplicative-zero on expst via
gpsimd affine_select (Pool engine), so the DVE stays out of the mask path.

Per-core dataflow:
  Phase A (per 128-row chunk g, software-pipelined):
    x chunk --PE transpose--> xT --mm--> q,k,v (natural); batched RoPE on DVE
    (broadcast cos/sin over the 5 q/k blocks); PE transpose q_rope/k_rope ->
    persistent qT[d,h,s], kT[d,s]; v natural -> vv[s,d].
  Attention (per 256-col tile t, head h, 2-stage pipelined; light doses
  interleave into phase A, the bulk runs after):
    scoresT[sk,sq] = kT_chunk.T @ qT ; exp on ACT -> expst f16 ; diagonal
    causal zeroing on Pool ; den = DoubleRow fp8 ones.T @ expst8 ; U^T
    accumulated fp16 ; rec = 1/den (DVE) ; uT = U^T * rec (DVE, f16).
    wo: y[g,:] = sum_h uT_h.T @ wo_h -> y_sb f16 -> DRAM (one DMA per row).
"""

import sys

import numpy as np

for _p in ("/opt/trn_rl_repo", "/root/.axon_site/_ro/trn_rl_repo"):
    if _p not in sys.path:
        sys.path.append(_p)

from contextlib import ExitStack

import concourse.bass as bass
import concourse.mybir as mybir
from concourse import bacc
from concourse.masks import make_identity
from concourse.tile import TileContext

P = 128           # partitions / head dim / seq chunk
S = 1024          # sequence length
HID = 2048        # model dim
NH = 4            # query heads per core
D = 128           # head dim
TQ = 256          # query macro-tile
NT = S // TQ      # 4 macro tiles
KC = HID // P     # 16 contraction chunks
NSK = S // P      # 8 key chunks
NG = S // P       # 8 row chunks
H2 = D // 2
F32 = mybir.dt.float32
F16 = mybir.dt.float16
F8 = mybir.dt.float8e4
SCALE = 1.0 / float(np.sqrt(D))
NEG = -30000.0
AL = mybir.AluOpType
AF = mybir.ActivationFunctionType
DR = mybir.MatmulPerfMode.DoubleRow

# fp8 expst is dead: exp values span e^{+-9.2}, outside e4m3 range (den can
# round to 0 -> 1/den=inf -> NaN; large values saturate at 448)
USE_F8 = False

N_CORES = 8
B = 2
N_KV = 4


def build_nc():
    nc = bacc.Bacc("TRN2", target_bir_lowering=False, debug=False)
    x_d = nc.declare_dram_parameter("x", [S, HID], F16, isOutput=False)
    cos_d = nc.declare_dram_parameter("cos", [S, D], F16, isOutput=False)
    sin_d = nc.declare_dram_parameter("sin", [S, D], F16, isOutput=False)
    wq_d = nc.declare_dram_parameter("wq", [HID, NH * D], F16, isOutput=False)
    wk_d = nc.declare_dram_parameter("wk", [HID, D], F16, isOutput=False)
    wv_d = nc.declare_dram_parameter("wv", [HID, D], F16, isOutput=False)
    wo_d = nc.declare_dram_parameter("wo", [NH * D, HID], F16, isOutput=False)
    out_d = nc.declare_dram_parameter("out", [S, HID], F16, isOutput=True)

    with TileContext(nc) as tc, ExitStack() as ctx:
        consts = ctx.enter_context(tc.tile_pool(name="consts", bufs=1))
        wpool = ctx.enter_context(tc.tile_pool(name="wpool", bufs=1))
        persist = ctx.enter_context(tc.tile_pool(name="persist", bufs=1))

        # ---- tile declarations (DMAs can start before consts are built) ----
        ident_f32 = consts.tile([P, P], F32, tag="ident_f32")
        ident = consts.tile([P, P], F16, tag="ident")
        ones8 = consts.tile([P, 2, P], F8, tag="ones8")
        ones16 = consts.tile([P, P], F16, tag="ones16")

        # ---- weights (partition-chunked layouts) ----
        wq_sb = wpool.tile([P, KC, NH * D], F16, tag="wq")
        wq_r = wq_d[:].rearrange("(c p) n -> p c n", p=P)
        wkv_sb = wpool.tile([P, KC, 2 * D], F16, tag="wkv")
        wo_sb = wpool.tile([P, NH, HID], F16, tag="wo")
        wo_r = wo_d[:].rearrange("(h p) n -> p h n", p=P)
        cos_sb = wpool.tile([P, NG, D], F16, tag="cos")
        sin_sb = wpool.tile([P, NG, D], F16, tag="sin")

        # persistent transposed activations
        qT_all = persist.tile([P, NH, S], F16, tag="qT")   # [d, h, sq]
        kT = persist.tile([P, S], F16, tag="kT")           # [d, sk]
        vv = persist.tile([P, NSK, D], F16, tag="vv")      # v natural [sk, d]

        # ---- SBUF working pools ----
        pa = ctx.enter_context(tc.tile_pool(name="pa", bufs=2))
        pb = ctx.enter_context(tc.tile_pool(name="pb", bufs=2))
        # ---- PSUM (8 banks): phase A uses qkv 2 + tp 2 (inner stack,
        # released before wo); s 2 + ud 2 persist; y 2 allocated after ----
        ps_s = ctx.enter_context(tc.tile_pool(name="ps_s", bufs=2, space="PSUM"))
        ps_ud = ctx.enter_context(tc.tile_pool(name="ps_ud", bufs=2, space="PSUM"))
        phase_a_ctx = ExitStack()
        ps_qkv = phase_a_ctx.enter_context(
            tc.tile_pool(name="ps_qkv", bufs=1, space="PSUM")
        )
        ps_tp = phase_a_ctx.enter_context(
            tc.tile_pool(name="ps_tp", bufs=2, space="PSUM")
        )

        x_tiles = [None] * NG
        x_r = x_d[:].rearrange("(c p) n -> p c n", p=P)

        def emit_xdma_pair(g, single=False):
            """One DMA per chunk pair (fewer per-DMA overheads)."""
            n = 1 if single else 2
            xp = pa.tile([P, n, HID], F16, tag=f"xnat{g}", bufs=1)
            nc.sync.dma_start(out=xp, in_=x_r[:, g : g + n, :])
            for j in range(n):
                x_tiles[g + j] = xp[:, j, :]

        # warm up the PE clock immediately — the stationary only needs a
        # cheap DVE memset, not the gpsimd-built identity
        warm_src = consts.tile([P, P], F16, tag="warmsrc")
        nc.vector.memset(warm_src, 1.0)
        warm_ps = ps_s.tile([P, 2 * TQ], F32, tag="s", name="warm")
        for _ in range(30):
            nc.tensor.matmul(
                warm_ps[:, 0:P], warm_src, warm_src, start=True, stop=True
            )
        warm_drain = pa.tile([P, 4], F32, tag="warmdrain", bufs=1)
        nc.vector.tensor_copy(warm_drain, warm_ps[:, 0:4])

        # Parallel DGE queues: x + wq on sync (SP, HWDGE — costs no engine
        # time); wkv/cos/sin on the scalar queue; Pool keeps only the const
        # builds (ident/m12) plus wo and output DMAs.
        emit_xdma_pair(0, single=True)
        emit_xdma_pair(1, single=True)
        nc.sync.dma_start(out=wq_sb[:, 0:8, :], in_=wq_r[:, 0:8, :])
        nc.scalar.dma_start(
            out=wkv_sb[:, :, 0:D], in_=wk_d[:].rearrange("(c p) n -> p c n", p=P)
        )
        nc.scalar.dma_start(
            out=wkv_sb[:, :, D : 2 * D],
            in_=wv_d[:].rearrange("(c p) n -> p c n", p=P),
        )
        nc.scalar.dma_start(
            out=cos_sb, in_=cos_d[:].rearrange("(c p) d -> p c d", p=P)
        )
        nc.scalar.dma_start(
            out=sin_sb, in_=sin_d[:].rearrange("(c p) d -> p c d", p=P)
        )

        # ---- constants (Pool engine is free: ident lands early) ----
        make_identity(nc, ident_f32)
        nc.vector.tensor_copy(ident, ident_f32)
        nc.vector.memset(ones8, 1.0)
        nc.vector.memset(ones16, 1.0)
        # causal masks for the two diagonal-straddling chunk positions
        m12 = consts.tile([P, 2 * TQ], F32, tag="m12")
        nc.gpsimd.memset(m12, 0.0)
        nc.gpsimd.affine_select(
            out=m12[:, 0:TQ], in_=m12[:, 0:TQ], compare_op=AL.is_ge, fill=NEG,
            base=0, pattern=[[1, TQ]], channel_multiplier=-1,
        )
        nc.gpsimd.affine_select(
            out=m12[:, TQ : 2 * TQ], in_=m12[:, TQ : 2 * TQ],
            compare_op=AL.is_ge, fill=NEG,
            base=-P, pattern=[[1, TQ]], channel_multiplier=-1,
        )

        nc.sync.dma_start(out=wq_sb[:, 8:16, :], in_=wq_r[:, 8:16, :])
        emit_xdma_pair(2)
        emit_xdma_pair(4)
        emit_xdma_pair(6)
        wo_next = [0]

        def emit_wo_dma():
            h = wo_next[0]
            if h < NH:
                nc.gpsimd.dma_start(
                    out=wo_sb[:, h : h + 2, :], in_=wo_r[:, h : h + 2, :]
                )
                wo_next[0] += 2

        def transposes(g):
            """x chunk -> xT (PE transpose, f16)."""
            x_nat = x_tiles[g]
            xT = pa.tile([P, KC, P], F16, tag="xT", bufs=2)
            xT_flat = xT.rearrange("p c d -> p (c d)")
            for kb in range(KC // 4):
                tp_ps = ps_tp.tile([P, 5 * P], F16, tag="tp", name="tp")
                for j in range(4):
                    k = 4 * kb + j
                    nc.tensor.transpose(
                        tp_ps[:, j * P : (j + 1) * P],
                        x_nat[:, k * P : (k + 1) * P],
                        ident,
                    )
                if kb % 2 == 0:
                    nc.vector.tensor_copy(
                        xT_flat[:, kb * 4 * P : (kb + 1) * 4 * P], tp_ps[:, 0 : 4 * P]
                    )
                else:
                    nc.scalar.activation(
                        out=xT_flat[:, kb * 4 * P : (kb + 1) * 4 * P],
                        in_=tp_ps[:, 0 : 4 * P],
                        func=AF.Copy,
                    )
            return xT

        def proj(g, xT):
            """q, k, v projections for chunk g (PE, accumulating in PSUM)."""
            qkv_ps = ps_qkv.tile([P, NH * D + 2 * D], F32, tag="qkv")
            q_ps = qkv_ps[:, 0 : NH * D]
            kv_ps = qkv_ps[:, NH * D : NH * D + 2 * D]
            for k in range(KC):
                nc.tensor.matmul(
                    q_ps, xT[:, k, :], wq_sb[:, k, :],
                    start=(k == 0), stop=(k == KC - 1),
                )
            for k in range(KC):
                nc.tensor.matmul(
                    kv_ps, xT[:, k, :], wkv_sb[:, k, :],
                    start=(k == 0), stop=(k == KC - 1),
                )
            # copy-out split across ACT (q) and DVE (kv) to free the bank fast
            qkv_sb = pa.tile([P, NH * D + 2 * D], F16, tag="qkvsb")
            nc.scalar.activation(
                out=qkv_sb[:, 0 : NH * D], in_=q_ps, func=AF.Copy
            )
            nc.vector.tensor_copy(qkv_sb[:, NH * D :], kv_ps)
            return qkv_sb

        def rope_stage(g, qkv_sb):
            """Batched RoPE over the 5 q/k blocks (DVE, broadcast cos/sin)."""
            qk = qkv_sb[:, 0 : 5 * D].rearrange("p (f d) -> p f d", d=D)
            sin_lo = sin_sb[:, g : g + 1, 0:H2].to_broadcast((P, 5, H2))
            sin_hi = sin_sb[:, g : g + 1, H2:D].to_broadcast((P, 5, H2))
            cos_bc = cos_sb[:, g : g + 1, :].to_broadcast((P, 5, D))
            tmp = pa.tile([P, 5, D], F16, tag="ropetmp")
            dst = pa.tile([P, 5, D], F16, tag="qkrope")
            nc.vector.scalar_tensor_tensor(
                out=tmp[:, :, 0:H2], in0=qk[:, :, H2:D], scalar=-1.0,
                in1=sin_lo, op0=AL.mult, op1=AL.mult,
            )
            nc.vector.tensor_tensor(
                out=tmp[:, :, H2:D], in0=qk[:, :, 0:H2], in1=sin_hi, op=AL.mult
            )
            nc.vector.tensor_tensor(out=dst, in0=qk, in1=cos_bc, op=AL.mult)
            nc.vector.tensor_tensor(
                out=dst.rearrange("p f d -> p (f d)"),
                in0=dst.rearrange("p f d -> p (f d)"),
                in1=tmp.rearrange("p f d -> p (f d)"),
                op=AL.add,
            )
            # v copy-out (cast f16)
            nc.vector.tensor_copy(vv[:, g, :], qkv_sb[:, 5 * D : 6 * D])
            return dst

        def rope_transpose(g, dst):
            """Transpose RoPE'd q/k into persistent qT_all / kT."""
            tq_ps = ps_tp.tile([P, 5 * P], F16, tag="tp", name="tq")
            for f in range(5):
                nc.tensor.transpose(
                    tq_ps[:, f * P : (f + 1) * P], dst[:, f, :], ident
                )
            nc.vector.tensor_copy(
                qT_all[:, :, g * P : (g + 1) * P],
                tq_ps[:, 0 : 4 * P].rearrange("p (h d) -> p h d", h=NH),
            )
            nc.scalar.activation(
                out=kT[:, g * P : (g + 1) * P], in_=tq_ps[:, 4 * P : 5 * P],
                func=AF.Copy,
            )

        ropes = [None] * NG
        pend = [None] * NG

        def emit_phase_a(g):
            if g >= 2:
                gg = g - 2
                with nc.named_scope(f"rope_{gg}"):
                    ropes[gg] = rope_stage(gg, pend[gg][1])
            if g < NG:
                if g in (2, 3):
                    emit_wo_dma()
                with nc.named_scope(f"tp_{g}"):
                    xT = transposes(g)
                pend[g] = [xT, None]
            if g >= 1 and g - 1 < NG:
                gg = g - 1
                with nc.named_scope(f"proj_{gg}"):
                    qkv_sb = proj(gg, pend[gg][0])
                pend[gg][1] = qkv_sb
            if g >= 2:
                gg = g - 2
                with nc.named_scope(f"ropeT_{gg}"):
                    rope_transpose(gg, ropes[gg])
                pend[gg] = None

        # ---------- attention ----------
        EDT = F8 if USE_F8 else F16

        def scores_head(t, h):
            """scoresT + causal mask (diagonal, DVE pre-exp) + exp -> expst.

            expst is fp8: the SAME quantized values feed both the PV
            numerator (mixed fp16xfp8 matmul) and the denominator (fp8
            DoubleRow), so quantization error cancels in the ratio."""
            qT_h = qT_all[:, h, t * TQ : (t + 1) * TQ]
            expst = pb.tile([P, NSK, TQ], EDT, tag="expst", bufs=3)
            expst_flat = expst.rearrange("p c f -> p (c f)")
            for pi in range(t + 1):
                s_ps = ps_s.tile([P, 2 * TQ], F32, tag="s", name="s")
                for half in range(2):
                    ik = 2 * pi + half
                    nc.tensor.matmul(
                        s_ps[:, half * TQ : (half + 1) * TQ],
                        kT[:, ik * P : (ik + 1) * P], qT_h,
                        start=True, stop=True,
                    )
                if pi == t:
                    nc.vector.tensor_tensor(
                        out=s_ps, in0=s_ps, in1=m12, op=AL.add
                    )
                nc.scalar.activation(
                    out=expst_flat[:, pi * 2 * TQ : (pi + 1) * 2 * TQ],
                    in_=s_ps, func=AF.Exp, scale=SCALE,
                )
            return expst

        def dnpv_head(t, h, expst, uT_t):
            """denominator + PV matmuls, then normalize into uT_t (DVE)."""
            nsk = 2 * (t + 1)
            ud_ps = ps_ud.tile([P, 2 * TQ], F32, tag="ud", name="ud")
            u_ps = ud_ps[:, 0:TQ]
            den_ps = ud_ps[:, TQ : 2 * TQ]
            if USE_F8:
                for pi in range(t + 1):
                    nc.tensor.matmul(
                        den_ps, ones8,
                        expst[:, 2 * pi : 2 * pi + 2, :],
                        start=(pi == 0), stop=(pi == t), perf_mode=DR,
                    )
            else:
                for ik in range(nsk):
                    nc.tensor.matmul(
                        den_ps, ones16, expst[:, ik, :],
                        start=(ik == 0), stop=(ik == nsk - 1),
                    )
            rec = pb.tile([P, TQ], F32, tag="rec", bufs=2)
            nc.vector.reciprocal(rec, den_ps)
            for ik in range(nsk):
                nc.tensor.matmul(
                    u_ps, vv[:, ik, :], expst[:, ik, :],
                    start=(ik == 0), stop=(ik == nsk - 1),
                )
            nc.vector.tensor_tensor(
                out=uT_t[:, h, :], in0=u_ps, in1=rec, op=AL.mult
            )

        ps_y_box = [None]

        def wo_stage(t, uT_t):
            for sub in range(2):
                g = 2 * t + sub
                y_sb = pb.tile([P, HID], F16, tag="ysb", bufs=2)
                for n in range(HID // 512):
                    y_ps = ps_y_box[0].tile([P, 512], F32, tag="y", name="y")
                    for h in range(NH):
                        nc.tensor.matmul(
                            y_ps,
                            uT_t[:, h, sub * P : (sub + 1) * P],
                            wo_sb[:, h, n * 512 : (n + 1) * 512],
                            start=(h == 0), stop=(h == NH - 1),
                        )
                    if n % 2 == 0:
                        nc.vector.tensor_copy(
                            y_sb[:, n * 512 : (n + 1) * 512], y_ps
                        )
                    else:
                        nc.scalar.activation(
                            out=y_sb[:, n * 512 : (n + 1) * 512], in_=y_ps,
                            func=AF.Copy,
                        )
                nc.gpsimd.dma_start(
                    out=out_d[g * P : (g + 1) * P, :], in_=y_sb
                )

        steps = [(t, h) for t in range(NT) for h in range(NH)]
        uts = {}
        att_i = [0]
        pending_wo = []

        def emit_attention_step(defer_wo):
            i = att_i[0]
            if i >= len(steps) + 2:
                return False
            if i < len(steps):
                t, h = steps[i]
                if h == 0:
                    uts[t] = pb.tile([P, NH, TQ], F16, tag="uT", name=f"uT{t}", bufs=4)
                with nc.named_scope(f"sc_{t}_{h}"):
                    uts[(t, h)] = scores_head(t, h)
            if 1 <= i < len(steps) + 1:
                t, h = steps[i - 1]
                with nc.named_scope(f"dnpv_{t}_{h}"):
                    dnpv_head(t, h, uts.pop((t, h)), uts[t])
            if i >= 2 and (i - 2) % NH == NH - 1:
                t = steps[i - 2][0]
                if defer_wo:
                    pending_wo.append(t)
                else:
                    while pending_wo:
                        tp = pending_wo.pop(0)
                        with nc.named_scope(f"wo_{tp}"):
                            wo_stage(tp, uts.pop(tp))
                    with nc.named_scope(f"wo_{t}"):
                        wo_stage(t, uts.pop(t))
            att_i[0] += 1
            return True

        def att_ready():
            i = att_i[0]
            if i >= len(steps) + 2:
                return False
            if i < len(steps):
                t, _h = steps[i]
                # one extra iteration of slack so the ropeT DVE copies of
                # the needed chunks have drained before the PE hits sc
                if 2 * t + 2 > done_g[0]:
                    return False
            return True

        # drive: phase A strictly prioritized; 1 attention step per
        # iteration to fill PE bubbles (wo deferred), the bulk after
        done_g = [-1]
        for g in range(NG + 2):
            emit_phase_a(g)
            done_g[0] = g - 2
            if g >= 3 and att_ready():
                emit_attention_step(defer_wo=True)
        # phase A fully emitted: release its PSUM banks, give wo its own
        phase_a_ctx.close()
        ps_y_box[0] = ctx.enter_context(
            tc.tile_pool(name="ps_y", bufs=2, space="PSUM")
        )
        while pending_wo and att_i[0] > 2:
            tp_ = pending_wo.pop(0)
            with nc.named_scope(f"wo_{tp_}"):
                wo_stage(tp_, uts.pop(tp_))
        while emit_attention_step(defer_wo=False):
            pass

    nc.compile()
    return nc


def shard_inputs(x, cos, sin, wq, wk, wv, wo):
    """Build per-core input maps (fp16): core = b*4 + g."""
    f16 = np.float16
    in_maps = []
    for c in range(N_CORES):
        b, g = divmod(c, N_KV)
        in_maps.append(
            {
                "x": np.ascontiguousarray(x[b], dtype=f16),
                "cos": np.ascontiguousarray(cos, dtype=f16),
                "sin": np.ascontiguousarray(sin, dtype=f16),
                "wq": np.ascontiguousarray(
                    wq[:, g * NH * D : (g + 1) * NH * D], dtype=f16
                ),
                "wk": np.ascontiguousarray(wk[:, g * D : (g + 1) * D], dtype=f16),
                "wv": np.ascontiguousarray(wv[:, g * D : (g + 1) * D], dtype=f16),
                "wo": np.ascontiguousarray(
                    wo[g * NH * D : (g + 1) * NH * D, :], dtype=f16
                ),
            }
        )
    return in_maps


_NC_CACHE = {}


def get_nc():
    if "nc" not in _NC_CACHE:
        _NC_CACHE["nc"] = build_nc()
    return _NC_CACHE["nc"]


def kernel(x, cos, sin, wq, wk, wv, wo, _trace=False):
    from concourse.bass_utils import run_bass_kernel_spmd

    x = np.asarray(x, dtype=np.float32)
    cos = np.asarray(cos, dtype=np.float32)
    sin = np.asarray(sin, dtype=np.float32)
    wq = np.asarray(wq, dtype=np.float32)
    wk = np.asarray(wk, dtype=np.float32)
    wv = np.asarray(wv, dtype=np.float32)
    wo = np.asarray(wo, dtype=np.float32)

    nc = get_nc()
    in_maps = shard_inputs(x, cos, sin, wq, wk, wv, wo)
    res = run_bass_kernel_spmd(nc, in_maps, list(range(N_CORES)), trace=_trace)
    parts = [
        np.asarray(res.results[c]["out"], dtype=np.float32) for c in range(N_CORES)
    ]
    y = np.stack(
        [sum(parts[b * N_KV + g] for g in range(N_KV)) for b in range(B)], axis=0
    )
    if _trace:
        kernel.last_result = res
    return y
